# revision 12
# baseline (speedup 1.0000x reference)
"""Expert-parallel MoE MLP kernel for Trainium2 (8 NeuronCores).

Problem: out[b,e,n,d] = gelu(x[b,e] @ w1[e] + b1[e]) @ w2[e] + b2[e]
Shapes: x [2,8,1024,1024] f32, w1 [8,1024,4096], b1 [8,4096],
        w2 [8,4096,1024], b2 [8,1024].

Sharding: expert e -> core e. Each core runs a 2048-token MLP:
  [2048,1024] @ [1024,4096] -> gelu -> @ [4096,1024] -> [2048,1024]

Device-side layout: activations live transposed ([feature, token]) so the
contraction dim is always the SBUF partition dim. Host transposes x on the
way in and out on the way back (part of shard/unshard), so the device does
zero transposes.

Precision strategy: fp8e4 (e4m3) matmuls in DoubleRow perf mode run the PE
at 2x rate (0.5 cycles/row). Each operand is split into an fp8 high part
plus an fp8 residual:
  x  ~= xh + xl          (unscaled; residual partly subnormal, abs err ~2^-10)
  64*w ~= wh + wl        (weights pre-scaled by 64 so they sit in e4m3's
                          normal range; residual stored unscaled)
and x@w is computed at scale 64 in PSUM as
  xh@wh + xh@wl + xl@wh      (xl@wl dropped, O(eps^2))
which is 3 DoubleRow passes = 0.75x the bf16 cycle count. The 1/64 descale
folds into the ScalarE activation (phase 1) / DVE tensor_scalar (phase 2)
that read PSUM anyway. End-to-end rel err ~2e-3 (slightly better than the
bf16 version's 3.4e-3).

Between layers the hidden activations are re-split on device: ScalarE
writes g = gelu(psum/64 + b1) in bf16, DVE derives hh = fp8(g) and
hl = fp8(g - hh). Phase-2 DoubleRow pairs adjacent h-tiles, so hh/hl live
in [P, 2, tblk] pair-tiles.
"""

import sys

for _p in ("/opt/trn_rl_repo",):
    if _p not in sys.path:
        sys.path.insert(0, _p)

import numpy as np
import ml_dtypes

from contextlib import ExitStack

import concourse.bass as bass
import concourse.tile as tile
from concourse import bacc, mybir
from concourse.bass_utils import run_bass_kernel_spmd

BF16 = mybir.dt.bfloat16
F32 = mybir.dt.float32
FP8 = mybir.dt.float8e4
NP_FP8 = ml_dtypes.float8_e4m3

# Full-problem constants (hardcoded per harness contract).
B, E, N, D, H = 2, 8, 1024, 1024, 4096
T = B * N          # tokens per expert/core
TBLK = 512         # tokens per block (= one PSUM bank of fp32)
P = 128
SW = 64.0          # weight pre-scale so 0.02-ish weights use e4m3 normals

DR = mybir.MatmulPerfMode.DoubleRow


def build_nc(t=T, d=D, h=H, tblk=TBLK, act=None, repeats=1,
             ps_bufs=4, act_mode="gelu", mm_mode="dr"):
    """Build the per-core Bass program. All cores run this same program on
    different data (SPMD). repeats>1 re-runs the token-block loop (weights
    stay resident) — used only for steady-state timing measurements.
    mm_mode: "dr" DoubleRow fp8 (2x PE rate) | "flat" plain fp8 matmuls."""
    if act is None:
        act = mybir.ActivationFunctionType.Gelu_apprx_tanh
    kd = d // P        # contraction tiles for phase 1
    nh = h // P        # h tiles (phase-1 outputs / phase-2 contraction)
    nd = d // P        # d tiles (phase-2 outputs)
    nblk = t // tblk
    assert kd % 2 == 0 and nh % 2 == 0

    nc = bacc.Bacc("TRN2", target_bir_lowering=False)

    xh_hbm = nc.dram_tensor("xh", [d, t], FP8, kind="ExternalInput").ap()
    xl_hbm = nc.dram_tensor("xl", [d, t], FP8, kind="ExternalInput").ap()
    # Weights arrive pre-packed tile-contiguously (host does the transpose)
    # so each tile DMA moves >=1KB contiguous runs per partition -- short
    # (<512B) descriptor runs pay a 2x DMA bandwidth penalty.
    w1h_hbm = nc.dram_tensor("w1h", [nh, P, kd, P], FP8,
                             kind="ExternalInput").ap()
    w1l_hbm = nc.dram_tensor("w1l", [nh, P, kd, P], FP8,
                             kind="ExternalInput").ap()
    w2h_hbm = nc.dram_tensor("w2h", [nh // 2, P, 2, d], FP8,
                             kind="ExternalInput").ap()
    w2l_hbm = nc.dram_tensor("w2l", [nh // 2, P, 2, d], FP8,
                             kind="ExternalInput").ap()
    b1_hbm = nc.dram_tensor("b1", [nh, P], F32, kind="ExternalInput").ap()
    b2_hbm = nc.dram_tensor("b2", [nd, P], F32, kind="ExternalInput").ap()
    out_hbm = nc.dram_tensor("outT", [d, t], F32, kind="ExternalOutput").ap()

    # [feature, x] views with the 128-partition dim innermost in features.
    xh_v = xh_hbm.rearrange("(kd p) t -> p kd t", p=P)
    xl_v = xl_hbm.rearrange("(kd p) t -> p kd t", p=P)

    with tile.TileContext(nc) as tc, ExitStack() as ctx:
        w1_pool = ctx.enter_context(tc.tile_pool(name="w1", bufs=2 * nh))
        w2_pool = ctx.enter_context(tc.tile_pool(name="w2", bufs=nh))
        x_pool = ctx.enter_context(tc.tile_pool(name="x", bufs=2))
        hh_pool = ctx.enter_context(tc.tile_pool(name="hh", bufs=1))
        hl_pool = ctx.enter_context(tc.tile_pool(name="hl", bufs=1))
        g_pool = ctx.enter_context(tc.tile_pool(name="g", bufs=4))
        o_pool = ctx.enter_context(tc.tile_pool(name="o", bufs=4))
        c_pool = ctx.enter_context(tc.tile_pool(name="c", bufs=1))
        ps1 = ctx.enter_context(tc.tile_pool(name="ps1", bufs=ps_bufs, space="PSUM"))
        ps2 = ctx.enter_context(tc.tile_pool(name="ps2", bufs=ps_bufs, space="PSUM"))

        # Biases, resident.
        b1_sb = c_pool.tile([P, nh], F32)
        nc.sync.dma_start(out=b1_sb, in_=b1_hbm.rearrange("t p -> p t"))
        b2_sb = c_pool.tile([P, nd], F32)
        nc.sync.dma_start(out=b2_sb, in_=b2_hbm.rearrange("t p -> p t"))

        # Weights, resident in SBUF for the whole kernel. Chunked DMAs so
        # compute can start as soon as the first chunks land.
        w1h_t, w1l_t = [], []
        for ih in range(nh):
            wt = w1_pool.tile([P, kd, P], FP8)
            nc.sync.dma_start(out=wt, in_=w1h_hbm[ih])
            w1h_t.append(wt)
            wt = w1_pool.tile([P, kd, P], FP8)
            nc.sync.dma_start(out=wt, in_=w1l_hbm[ih])
            w1l_t.append(wt)
        w2h_t, w2l_t = [], []
        for j in range(nh // 2):
            wt = w2_pool.tile([P, 2, d], FP8)
            nc.sync.dma_start(out=wt, in_=w2h_hbm[j])
            w2h_t.append(wt)
            wt = w2_pool.tile([P, 2, d], FP8)
            nc.sync.dma_start(out=wt, in_=w2l_hbm[j])
            w2l_t.append(wt)

        MM = nc.tensor.matmul
        ts_mult = mybir.AluOpType.mult
        ts_add = mybir.AluOpType.add
        ts_sub = mybir.AluOpType.subtract

        for ib in [i % nblk for i in range(nblk * repeats)]:
            tsl = slice(ib * tblk, (ib + 1) * tblk)
            # gpsimd (SWDGE) queue: keeps x blocks off the sync queue so they
            # don't sit behind 16 MiB of weight DMAs at startup.
            xh_sb = x_pool.tile([P, kd, tblk], FP8)
            nc.gpsimd.dma_start(out=xh_sb, in_=xh_v[:, :, tsl])
            xl_sb = x_pool.tile([P, kd, tblk], FP8)
            nc.gpsimd.dma_start(out=xl_sb, in_=xl_v[:, :, tsl])

            # phase 1: hT[h_tile] = gelu((xh+xl) @ (w1h+w1l)/64 + b1)
            hh_t, hl_t = [], []
            for ih in range(nh):
                j, c = divmod(ih, 2)
                if c == 0:
                    hh_t.append(hh_pool.tile([P, 2, tblk], FP8,
                                             name=f"hh{j}", tag=f"hh{j}"))
                    hl_t.append(hl_pool.tile([P, 2, tblk], FP8,
                                             name=f"hl{j}", tag=f"hl{j}"))
                ps = ps1.tile([P, tblk], F32)
                if mm_mode == "dr":
                    nk = kd // 2
                    for m in range(nk):
                        sl = slice(2 * m, 2 * m + 2)
                        MM(ps, w1h_t[ih][:, sl, :], xh_sb[:, sl, :],
                           start=(m == 0), stop=False, perf_mode=DR)
                        MM(ps, w1l_t[ih][:, sl, :], xh_sb[:, sl, :],
                           start=False, stop=False, perf_mode=DR)
                        MM(ps, w1h_t[ih][:, sl, :], xl_sb[:, sl, :],
                           start=False, stop=(m == nk - 1), perf_mode=DR)
                else:
                    for m in range(kd):
                        MM(ps, w1h_t[ih][:, m, :], xh_sb[:, m, :],
                           start=(m == 0), stop=False)
                        MM(ps, w1l_t[ih][:, m, :], xh_sb[:, m, :],
                           start=False, stop=False)
                        MM(ps, w1h_t[ih][:, m, :], xl_sb[:, m, :],
                           start=False, stop=(m == kd - 1))
                g = g_pool.tile([P, tblk], BF16)
                if act_mode == "gelu":
                    nc.scalar.activation(g, ps, act,
                                         bias=b1_sb[:, ih:ih + 1], scale=1.0 / SW)
                else:
                    nc.scalar.activation(
                        g, ps, mybir.ActivationFunctionType.Copy,
                        bias=b1_sb[:, ih:ih + 1], scale=1.0 / SW)
                hh = hh_t[j][:, c, :]
                hl = hl_t[j][:, c, :]
                nc.vector.tensor_copy(hh, g)
                nc.vector.scalar_tensor_tensor(hl, g, 1.0, hh, ts_mult, ts_sub)

            # phase 2: outT[d_tile] = (hh+hl) @ (w2h+w2l)/64 + b2
            for idt in range(nd):
                csl = slice(idt * P, (idt + 1) * P)
                ps = ps2.tile([P, tblk], F32)
                nj = nh // 2
                if mm_mode == "dr":
                    for j in range(nj):
                        MM(ps, w2h_t[j][:, :, csl], hh_t[j],
                           start=(j == 0), stop=False, perf_mode=DR)
                        MM(ps, w2l_t[j][:, :, csl], hh_t[j],
                           start=False, stop=False, perf_mode=DR)
                        MM(ps, w2h_t[j][:, :, csl], hl_t[j],
                           start=False, stop=(j == nj - 1), perf_mode=DR)
                else:
                    for j in range(nj):
                        for r in range(2):
                            MM(ps, w2h_t[j][:, r, csl], hh_t[j][:, r, :],
                               start=(j == 0 and r == 0), stop=False)
                            MM(ps, w2l_t[j][:, r, csl], hh_t[j][:, r, :],
                               start=False, stop=False)
                            MM(ps, w2h_t[j][:, r, csl], hl_t[j][:, r, :],
                               start=False,
                               stop=(j == nj - 1 and r == 1))
                ob = o_pool.tile([P, tblk], F32)
                nc.vector.tensor_scalar(ob, ps, 1.0 / SW, b2_sb[:, idt:idt + 1],
                                        ts_mult, ts_add)
                nc.scalar.dma_start(out=out_hbm[csl, tsl], in_=ob)

    nc.compile()
    return nc


_NC_CACHE = {}


def _get_nc():
    if "nc" not in _NC_CACHE:
        _NC_CACHE["nc"] = build_nc()
    return _NC_CACHE["nc"]


def _split_fp8(a):
    """a (f32) -> (hi, lo) fp8e4 with hi + lo ~= a."""
    hi = a.astype(NP_FP8)
    lo = (a - hi.astype(np.float32)).astype(NP_FP8)
    return hi, lo


def _pack_w1(a, kd, nh):
    """[D, H] -> tile-contiguous [nh, P, kd, P]."""
    return np.ascontiguousarray(
        a.reshape(kd, P, nh, P).transpose(2, 1, 0, 3))


def _pack_w2(a, nh, d):
    """[H, D] -> pair-tile-contiguous [nh//2, P, 2, D]."""
    return np.ascontiguousarray(
        a.reshape(nh // 2, 2, P, d).transpose(0, 2, 1, 3))


def make_in_map(xe, w1e, b1e, w2e, b2e):
    """Build the per-core input map from one expert's f32 slices.
    xe: [T, D]; w1e: [D, H]; b1e: [H]; w2e: [H, D]; b2e: [D]."""
    xt = np.ascontiguousarray(np.asarray(xe, np.float32).T)
    xh, xl = _split_fp8(xt)
    w1h, w1l = _split_fp8(np.asarray(w1e, np.float32) * SW)
    w2h, w2l = _split_fp8(np.asarray(w2e, np.float32) * SW)
    d, h = w1e.shape
    kd, nh = d // P, h // P
    return {
        "xh": xh, "xl": xl,
        "w1h": _pack_w1(w1h, kd, nh), "w1l": _pack_w1(w1l, kd, nh),
        "w2h": _pack_w2(w2h, nh, d), "w2l": _pack_w2(w2l, nh, d),
        "b1": np.ascontiguousarray(
            np.asarray(b1e, np.float32).reshape(h // P, P)),
        "b2": np.ascontiguousarray(
            np.asarray(b2e, np.float32).reshape(d // P, P)),
    }


def kernel(x, w1, b1, w2, b2):
    nc = _get_nc()
    in_maps = []
    for e in range(E):
        xe = np.asarray(x[:, e], dtype=np.float32).reshape(T, D)
        in_maps.append(make_in_map(xe, w1[e], b1[e], w2[e], b2[e]))

    res = run_bass_kernel_spmd(nc, in_maps, core_ids=list(range(E)))

    out = np.empty((B, E, N, D), dtype=np.float32)
    for e in range(E):
        ot = np.asarray(res.results[e]["outT"])            # [D, T]
        out[:, e] = ot.T.reshape(B, N, D)
    return out


# revision 20
# speedup vs baseline: 2.4374x; 2.4374x over previous
"""Expert-parallel MoE MLP kernel for Trainium2 (8 NeuronCores).

Problem: out[b,e,n,d] = gelu(x[b,e] @ w1[e] + b1[e]) @ w2[e] + b2[e]
Shapes: x [2,8,1024,1024] f32, w1 [8,1024,4096], b1 [8,4096],
        w2 [8,4096,1024], b2 [8,1024].

Sharding: expert e -> core e. Each core runs a 2048-token MLP:
  [2048,1024] @ [1024,4096] -> gelu -> @ [4096,1024] -> [2048,1024]

Device-side layout: activations live transposed ([feature, token]) so the
contraction dim is always the SBUF partition dim:
  phase 1: psum[h_tile, t] += w1[d_tile, h_tile].T @ xT[d_tile, t]
  phase 2: psum[d_tile, t] += w2[h_tile, d_tile].T @ hT[h_tile, t]
Host transposes x on the way in and out on the way back (part of
shard/unshard), so the device does zero transposes.

All matmul inputs are bf16 (fp32 PSUM accumulation); GELU (tanh approx,
matching jax.nn.gelu default) fused with the b1 add on ScalarE.

Notes from measurement (fp8 experiments abandoned): fp8 e4m3 DoubleRow /
DoubleRowSwInterleave matmuls run the PE stream at 2x, but on real TRN2 the
256-row stationary loads serialize against the 256-cycle streams (and plain
DoubleRow intermittently wedges the PE), so the 1.5x instruction count of
the split-precision scheme nets out slower than plain bf16 (756-883us vs
~508us per pass). bf16 keeps ldweights (128 rows) hidden under 512-cycle
streams and is stable.

w1 arrives tile-contiguous ([nh, P, kd*P], host-packed) so each weight-tile
DMA moves 2KB contiguous runs per partition: sub-512B descriptor runs pay a
2x DMA bandwidth penalty, which matters for how fast the first block's
weights land (startup is weight-DMA limited). x streams per 512-token block
through the gpsimd (SWDGE) queue so it never queues behind the 16 MiB of
weight DMAs on the sync queue.
"""

import sys

for _p in ("/opt/trn_rl_repo",):
    if _p not in sys.path:
        sys.path.insert(0, _p)

import numpy as np
import ml_dtypes

from contextlib import ExitStack

import concourse.tile as tile
from concourse import bacc, mybir
from concourse.bass_utils import run_bass_kernel_spmd

BF16 = mybir.dt.bfloat16
F32 = mybir.dt.float32

# Full-problem constants (hardcoded per harness contract).
B, E, N, D, H = 2, 8, 1024, 1024, 4096
T = B * N          # tokens per expert/core
TBLK = 512         # tokens per block (= one PSUM bank of fp32)
P = 128


def build_nc(t=T, d=D, h=H, tblk=TBLK, act=None, repeats=1,
             ps_bufs=4, act_mode="gelu", x_mode="stream"):
    """Build the per-core Bass program. All cores run this same program on
    different data (SPMD). repeats>1 re-runs the token-block loop (weights
    stay resident) — used only for steady-state timing measurements."""
    if act is None:
        act = mybir.ActivationFunctionType.Gelu_apprx_tanh
    kd = d // P        # contraction tiles for phase 1
    nh = h // P        # h tiles (phase-1 outputs / phase-2 contraction)
    nd = d // P        # d tiles (phase-2 outputs)
    nblk = t // tblk

    nc = bacc.Bacc("TRN2", target_bir_lowering=False)

    xt_hbm = nc.dram_tensor("xt", [d, t], BF16, kind="ExternalInput").ap()
    # w1 pre-packed tile-contiguous on host: [nh, P, kd, P]
    w1_hbm = nc.dram_tensor("w1", [nh, P, kd, P], BF16,
                            kind="ExternalInput").ap()
    w2_hbm = nc.dram_tensor("w2", [h, d], BF16, kind="ExternalInput").ap()
    b1_hbm = nc.dram_tensor("b1", [nh, P], F32, kind="ExternalInput").ap()
    b2_hbm = nc.dram_tensor("b2", [nd, P], F32, kind="ExternalInput").ap()
    out_hbm = nc.dram_tensor("outT", [d, t], F32, kind="ExternalOutput").ap()

    # [feature, x] views with the 128-partition dim innermost in features.
    xt_v = xt_hbm.rearrange("(kd p) t -> p kd t", p=P)
    w2_v = w2_hbm.rearrange("(kh p) d -> p kh d", p=P)

    with tile.TileContext(nc) as tc, ExitStack() as ctx:
        w1_pool = ctx.enter_context(tc.tile_pool(name="w1", bufs=nh))
        w2_pool = ctx.enter_context(tc.tile_pool(name="w2", bufs=nh))
        x_pool = ctx.enter_context(tc.tile_pool(name="x", bufs=2))
        h_pool = ctx.enter_context(tc.tile_pool(name="h", bufs=nh + 2))
        o_pool = ctx.enter_context(tc.tile_pool(name="o", bufs=4))
        c_pool = ctx.enter_context(tc.tile_pool(name="c", bufs=1))
        ps1 = ctx.enter_context(tc.tile_pool(name="ps1", bufs=ps_bufs, space="PSUM"))
        ps2 = ctx.enter_context(tc.tile_pool(name="ps2", bufs=ps_bufs, space="PSUM"))

        # Biases, resident.
        b1_sb = c_pool.tile([P, nh], F32)
        nc.sync.dma_start(out=b1_sb, in_=b1_hbm.rearrange("t p -> p t"))
        b2_sb = c_pool.tile([P, nd], F32)
        nc.sync.dma_start(out=b2_sb, in_=b2_hbm.rearrange("t p -> p t"))

        # Weights, resident in SBUF for the whole kernel. Chunked DMAs so
        # compute can start as soon as the first chunks land.
        w1_t = []
        for ih in range(nh):
            wt = w1_pool.tile([P, kd, P], BF16)
            nc.sync.dma_start(out=wt, in_=w1_hbm[ih])
            w1_t.append(wt)
        w2_t = []
        for ikh in range(nh):
            wt = w2_pool.tile([P, d], BF16)
            nc.sync.dma_start(out=wt, in_=w2_v[:, ikh, :])
            w2_t.append(wt)

        MM = nc.tensor.matmul

        xt_pre = {}
        if x_mode == "preload":
            for ib in range(nblk):
                xt_pre[ib] = c_pool.tile([P, kd, tblk], BF16,
                                         name=f"xp{ib}", tag=f"xp{ib}")
                nc.gpsimd.dma_start(
                    out=xt_pre[ib],
                    in_=xt_v[:, :, ib * tblk:(ib + 1) * tblk])
        for ib in [i % nblk for i in range(nblk * repeats)]:
            tsl = slice(ib * tblk, (ib + 1) * tblk)
            if x_mode == "preload":
                xt = xt_pre[ib]
            else:
                # gpsimd (SWDGE) queue: keeps x off the weight-DMA queue.
                xt = x_pool.tile([P, kd, tblk], BF16)
                nc.gpsimd.dma_start(out=xt, in_=xt_v[:, :, tsl])

            # phase 1: hT[h_tile] = gelu(w1.T @ xT + b1)
            ht = []
            for ih in range(nh):
                ps = ps1.tile([P, tblk], F32)
                for ik in range(kd):
                    MM(
                        ps, w1_t[ih][:, ik, :], xt[:, ik, :],
                        start=(ik == 0), stop=(ik == kd - 1),
                    )
                hs = h_pool.tile([P, tblk], BF16)
                if act_mode == "gelu":
                    nc.scalar.activation(hs, ps, act, bias=b1_sb[:, ih:ih + 1])
                else:
                    nc.vector.tensor_copy(hs, ps)
                ht.append(hs)

            # phase 2: outT[d_tile] = w2.T @ hT + b2
            for idt in range(nd):
                ps = ps2.tile([P, tblk], F32)
                for ikh in range(nh):
                    MM(
                        ps, w2_t[ikh][:, idt * P:(idt + 1) * P], ht[ikh],
                        start=(ikh == 0), stop=(ikh == nh - 1),
                    )
                ob = o_pool.tile([P, tblk], F32)
                nc.vector.tensor_scalar_add(ob, ps, b2_sb[:, idt:idt + 1])
                nc.scalar.dma_start(
                    out=out_hbm[idt * P:(idt + 1) * P, tsl], in_=ob
                )

    nc.compile()
    return nc


_NC_CACHE = {}


def _get_nc():
    if "nc" not in _NC_CACHE:
        _NC_CACHE["nc"] = build_nc()
    return _NC_CACHE["nc"]


def _pack_w1(a, kd, nh):
    """[D, H] bf16 -> tile-contiguous [nh, P, kd, P]."""
    return np.ascontiguousarray(
        a.reshape(kd, P, nh, P).transpose(2, 1, 0, 3))


def make_in_map(xe, w1e, b1e, w2e, b2e):
    """Build the per-core input map from one expert's f32 slices.
    xe: [T, D]; w1e: [D, H]; b1e: [H]; w2e: [H, D]; b2e: [D]."""
    bf16 = ml_dtypes.bfloat16
    d, h = w1e.shape
    kd, nh = d // P, h // P
    xtb = np.ascontiguousarray(np.asarray(xe, np.float32).T).astype(bf16)
    return {
        "xt": xtb,
        "w1": _pack_w1(np.asarray(w1e, np.float32).astype(bf16), kd, nh),
        "w2": np.asarray(w2e, np.float32).astype(bf16),
        "b1": np.ascontiguousarray(
            np.asarray(b1e, np.float32).reshape(h // P, P)),
        "b2": np.ascontiguousarray(
            np.asarray(b2e, np.float32).reshape(d // P, P)),
    }


def kernel(x, w1, b1, w2, b2):
    nc = _get_nc()
    in_maps = []
    for e in range(E):
        xe = np.asarray(x[:, e], dtype=np.float32).reshape(T, D)
        in_maps.append(make_in_map(xe, w1[e], b1[e], w2[e], b2[e]))

    res = run_bass_kernel_spmd(nc, in_maps, core_ids=list(range(E)))

    out = np.empty((B, E, N, D), dtype=np.float32)
    for e in range(E):
        ot = np.asarray(res.results[e]["outT"])            # [D, T]
        out[:, e] = ot.T.reshape(B, N, D)
    return out
